# revision 6
# baseline (speedup 1.0000x reference)
"""Trainium2 Bass kernel for a dense transformer block (QKV+gate proj, RoPE,
QK-RMSNorm, causal SDPA, output-RMSNorm + SiLU gate, output projection).

Sharding: tensor-parallel over heads across 8 NeuronCores (2 heads/core) for
projections+attention; token-parallel output projection with replicated Wo,
exchanged via 4 small AllToAlls (one per batch-half).

Schedule (engine FIFOs execute in trace order, so phases are interleaved):
  sweep A (q,k + fused rope/rms post)
  sweep B pairs 0,1  (gate+v for batch 0)
  attention b0 half0 + A2A, half1 + A2A     <- overlaps sweep-B b1 below
  sweep B pairs 2,3  (gate+v for batch 1)
  attention b1 half0 + A2A + finals(b0), half1 + A2A + finals(b1)
Attention is software-pipelined (scores i2+1 issued before PV i2) and each
(qb,m) tail's PSUM matmul is deferred behind the next iteration's scores.

Numerics: all matmuls bf16 (f32 accumulate). The output-RMSNorm eps term is
dropped (validated <3e-3 output error), removing all softmax-denominator
matmuls. rsqrt and silu are computed via exp/ln so the whole kernel uses a
single ACT table set (natural_log_exp_and_others).
"""

import os
import sys

for _p in ("/opt/trn_rl_repo", "/root/.axon_site/_ro/trn_rl_repo"):
    if os.path.isdir(_p) and _p not in sys.path:
        sys.path.insert(0, _p)

import numpy as np

import concourse.bass as bass
import concourse.mybir as mybir
from concourse import bacc
from concourse.bass_utils import run_bass_kernel_spmd
from concourse.tile import TileContext

B, T, HID = 2, 2048, 2048
H, D = 16, 128
NCORES = 8
HC = H // NCORES          # heads per core = 2
DC = HC * D               # 256 head-dims per core
BT = B * T                # 4096 tokens
KT = HID // 128           # 16 contraction tiles
SCALE = 1.0 / float(np.sqrt(D))
NEG = -3.0e38

F32 = mybir.dt.float32
BF16 = mybir.dt.bfloat16
AF = mybir.ActivationFunctionType
ALU = mybir.AluOpType

MMDT = BF16

LAST_EXEC_TIME_NS = None
_CACHED_NC = None


class _Bacc(bacc.Bacc):
    """Bacc with an ACT-table-set preference: serve Exp and Ln from the
    combined natural_log_exp_and_others set so alternating Ln/Exp chains
    don't thrash table loads."""

    def insert_act_table_loads(self):
        import bass_rust as _bass_rust
        from concourse.hw_specs import get_activation_tables
        has_activation = any(
            isinstance(i, mybir.InstActivation)
            for b in self.main_func.blocks
            for i in b.instructions
        )
        if not has_activation:
            return
        AFT = mybir.ActivationFunctionType
        tables = []
        for name, fns in get_activation_tables(self.m.arch).items():
            if name != "natural_log_exp_and_others":
                fns = fns - {AFT.Exp, AFT.Ln}
            tables.append((name, fns))
        _bass_rust.insert_act_table_loads(self, tables)


def _build_nc():
    nc = _Bacc("TRN2", target_bir_lowering=False, debug=False,
               num_devices=NCORES)

    xT = nc.dram_tensor("xT", [KT, BT // 1024, 128, 1024], MMDT,
                        kind="ExternalInput").ap()
    wq = nc.dram_tensor("wq", [128, KT, DC], MMDT, kind="ExternalInput").ap()
    wk = nc.dram_tensor("wk", [128, KT, DC], MMDT, kind="ExternalInput").ap()
    wv = nc.dram_tensor("wv", [128, KT, DC], MMDT, kind="ExternalInput").ap()
    wg = nc.dram_tensor("wg", [128, KT, DC], MMDT, kind="ExternalInput").ap()
    # full (replicated) Wo with o_norm folded: [128, kd-tile, 2048 cols]
    wo = nc.dram_tensor("wo", [128, KT, HID], MMDT, kind="ExternalInput").ap()
    cos2 = nc.dram_tensor("cos2", [128, T], F32, kind="ExternalInput").ap()
    sin2 = nc.dram_tensor("sin2", [128, T], F32, kind="ExternalInput").ap()
    negm = nc.dram_tensor("negm", [128, 128], F32, kind="ExternalInput").ap()
    ones_in = nc.dram_tensor("ones_in", [128, 128], MMDT,
                             kind="ExternalInput").ap()
    qrw = nc.dram_tensor("qrw", [128, 1], F32, kind="ExternalInput").ap()
    krw = nc.dram_tensor("krw", [128, 1], F32, kind="ExternalInput").ap()

    # output: per (b, half) a [128 tok, 2048] f32 chunk
    out_tok = nc.dram_tensor("out_tok", [B * 2, 128, HID], F32,
                             kind="ExternalOutput").ap()

    # AllToAll buffers: per (b, half): [8 shards(128 tok), 256 hd, 128 tok]
    a2a_in = [nc.dram_tensor(f"a2a_in{i}", [NCORES, DC, 128], MMDT).ap()
              for i in range(B * 2)]
    a2a_out = [nc.dram_tensor(f"a2a_out{i}", [NCORES, DC, 128], MMDT).ap()
               for i in range(B * 2)]

    tc = TileContext(nc)
    from contextlib import ExitStack
    stack = ExitStack()
    with tc:
        const = stack.enter_context(tc.tile_pool(name="const", bufs=1))
        ones_r = const.tile([128, 128], MMDT)
        nc.scalar.dma_start(out=ones_r, in_=ones_in)
        negm_sb = const.tile([128, 128], F32)
        nc.scalar.dma_start(out=negm_sb, in_=negm)
        onesb = const.tile([128, 1], F32)
        nc.vector.memset(onesb, 1.0)

        persist = stack.enter_context(tc.tile_pool(name="persist", bufs=1))
        qTf = [persist.tile([128, BT], MMDT, tag=f"qTf{m}", name=f"qTf{m}")
               for m in range(HC)]
        kTf = [persist.tile([128, BT], MMDT, tag=f"kTf{m}", name=f"kTf{m}")
               for m in range(HC)]
        v_sb = [persist.tile([128, BT // 128, 128], MMDT, tag=f"v{m}",
                             name=f"v{m}") for m in range(HC)]
        sg_sb = [persist.tile([128, BT], MMDT, tag=f"sg{m}", name=f"sg{m}")
                 for m in range(HC)]

        # ---------------- sweep A: q, k + fused post --------------------
        def sweep_a():
            with tc.tile_pool(name="swA_x", bufs=5) as xpool, \
                 tc.tile_pool(name="swA_ps", bufs=1, space="PSUM") as pps, \
                 tc.tile_pool(name="swA_ss", bufs=2, space="PSUM") as pss, \
                 tc.tile_pool(name="swA_t", bufs=1) as tpool, \
                 tc.tile_pool(name="ropec", bufs=1) as rp:
                cos_sb = rp.tile([128, T], F32)
                nc.sync.dma_start(out=cos_sb, in_=cos2)
                sin_sb = rp.tile([128, T], F32)
                nc.sync.dma_start(out=sin_sb, in_=sin2)
                qrw_sb = rp.tile([128, 1], F32)
                nc.sync.dma_start(out=qrw_sb, in_=qrw)
                krw_sb = rp.tile([128, 1], F32)
                nc.sync.dma_start(out=krw_sb, in_=krw)
                wsc = [qrw_sb, krw_sb]

                for nbb in range(BT // 1024):
                    xch = []
                    for c in range(4):
                        xc = xpool.tile([128, 4, 1024], MMDT, tag="xc",
                                        name="xc")
                        nc.sync.dma_start(
                            out=xc,
                            in_=xT[4 * c:4 * c + 4, nbb, :, :].rearrange(
                                "k p t -> p k t"))
                        xch.append(xc)
                    for half in range(2):
                        nb = 2 * nbb + half
                        c0 = nb * 512
                        ct0 = c0 % T
                        hs = slice(half * 512, (half + 1) * 512)
                        mms = [(mi, m) for mi in range(2) for m in range(HC)]
                        ps = {}
                        for mi, m in mms:
                            ps[(mi, m)] = pps.tile(
                                [128, 512], F32, tag=f"pp{mi}{m}",
                                name=f"pp{mi}{m}",
                                bufs=(2 if mi == 0 else 1))
                        for k in range(KT):
                            xk = xch[k // 4][:, k % 4, hs]
                            for mi, wn in enumerate(("q", "k")):
                                for m in range(HC):
                                    nc.tensor.matmul(
                                        ps[(mi, m)],
                                        wsb[wn][:, k, m * 128:(m + 1) * 128],
                                        xk, start=(k == 0),
                                        stop=(k == KT - 1))
                        dests = [qTf, kTf]
                        for mi, m in mms:
                            nc.vector.tensor_copy(
                                dests[mi][m][:, c0:c0 + 512], ps[(mi, m)])
                        # post: rms factor + rope in place; long-dependency
                        # ops go at the END of the DVE stream
                        raws = {k2: dests[k2[0]][k2[1]][:, c0:c0 + 512]
                                for k2 in mms}
                        facs, ros = {}, {}
                        for mi, m in mms:
                            raw = raws[(mi, m)]
                            sq = tpool.tile([128, 512], MMDT, tag="sq",
                                            name="sq", bufs=4)
                            nc.vector.tensor_mul(sq, raw, raw)
                            ss = pss.tile([128, 512], F32, tag="ss",
                                          name="ss")
                            nc.tensor.matmul(ss, ones_r, sq,
                                             start=True, stop=True)
                            lnt = tpool.tile([128, 512], F32, tag="lnt",
                                             name="lnt", bufs=4)
                            nc.scalar.activation(out=lnt, in_=ss, func=AF.Ln,
                                                 scale=1.0 / float(D))
                            fac = tpool.tile([128, 512], F32, tag="fac",
                                             name="fac", bufs=4)
                            nc.scalar.activation(out=fac, in_=lnt,
                                                 func=AF.Exp, scale=-0.5)
                            facs[(mi, m)] = fac
                        cc = cos_sb[:, ct0:ct0 + 512]
                        ssn = sin_sb[:, ct0:ct0 + 512]
                        for mi, m in mms:
                            raw = raws[(mi, m)]
                            # sin_sb top half pre-negated: full-width rope
                            sw = tpool.tile([128, 512], MMDT, tag="sw",
                                            name="sw", bufs=4)
                            nc.scalar.dma_start(out=sw[0:64, :],
                                                in_=raw[64:128, :])
                            nc.scalar.dma_start(out=sw[64:128, :],
                                                in_=raw[0:64, :])
                            u = tpool.tile([128, 512], F32, tag="u",
                                           name="u", bufs=4)
                            w2 = tpool.tile([128, 512], F32, tag="w2",
                                            name="w2", bufs=4)
                            nc.vector.tensor_mul(u, raw, cc)
                            nc.gpsimd.tensor_mul(w2, sw, ssn)
                            ro = tpool.tile([128, 512], F32, tag="ro",
                                            name="ro", bufs=4)
                            nc.gpsimd.tensor_add(ro, u, w2)
                            ros[(mi, m)] = ro
                        for mi, m in mms:
                            nc.vector.scalar_tensor_tensor(
                                out=raws[(mi, m)], in0=ros[(mi, m)],
                                scalar=wsc[mi], in1=facs[(mi, m)],
                                op0=ALU.mult, op1=ALU.mult)

        # ---------------- sweep B (gate, v) for a pair list -------------
        def sweep_b(nbb_list):
            with tc.tile_pool(name="swB_x", bufs=5) as xpool, \
                 tc.tile_pool(name="swB_ps", bufs=1, space="PSUM") as pps, \
                 tc.tile_pool(name="swB_t", bufs=1) as tpool:
                for nbb in nbb_list:
                    xch = []
                    for c in range(4):
                        xc = xpool.tile([128, 4, 1024], MMDT, tag="xc",
                                        name="xc")
                        nc.sync.dma_start(
                            out=xc,
                            in_=xT[4 * c:4 * c + 4, nbb, :, :].rearrange(
                                "k p t -> p k t"))
                        xch.append(xc)
                    # pair-interleaved: one weight load feeds both halves
                    ps = {}
                    for mi in range(2):
                        for m in range(HC):
                            for half in range(2):
                                ps[(mi, m, half)] = pps.tile(
                                    [128, 512], F32, tag=f"pp{mi}{m}{half}",
                                    name=f"pp{mi}{m}{half}", bufs=1)
                    for k in range(KT):
                        for mi, wn in enumerate(("g", "v")):
                            for m in range(HC):
                                for half in range(2):
                                    hs = slice(half * 512, (half + 1) * 512)
                                    nc.tensor.matmul(
                                        ps[(mi, m, half)],
                                        wsb[wn][:, k, m * 128:(m + 1) * 128],
                                        xch[k // 4][:, k % 4, hs],
                                        start=(k == 0), stop=(k == KT - 1))
                    # all evacuations first, ACT-dependent chains after
                    vflat = {}
                    graws = {}
                    for half in range(2):
                        c0 = (2 * nbb + half) * 512
                        for m in range(HC):
                            graw = tpool.tile([128, 512], F32, tag="graw",
                                              name="graw", bufs=6)
                            nc.vector.tensor_copy(graw, ps[(0, m, half)])
                            graws[(half, m)] = graw
                            vf = tpool.tile([128, 512], MMDT, tag="vf",
                                            name="vf", bufs=6)
                            nc.vector.tensor_copy(vf, ps[(1, m, half)])
                            vflat[(half, m)] = vf
                    for half in range(2):
                        nb = 2 * nbb + half
                        c0 = nb * 512
                        for m in range(HC):
                            graw = graws[(half, m)]
                            e1 = tpool.tile([128, 512], F32, tag="e1",
                                            name="e1", bufs=3)
                            nc.scalar.activation(out=e1, in_=graw,
                                                 func=AF.Exp, scale=-1.0)
                            l1 = tpool.tile([128, 512], F32, tag="l1",
                                            name="l1", bufs=3)
                            nc.scalar.activation(out=l1, in_=e1, func=AF.Ln,
                                                 scale=1.0, bias=onesb)
                            s1 = tpool.tile([128, 512], F32, tag="s1",
                                            name="s1", bufs=3)
                            nc.scalar.activation(out=s1, in_=l1,
                                                 func=AF.Exp, scale=-1.0)
                            nc.vector.tensor_mul(
                                sg_sb[m][:, c0:c0 + 512], graw, s1)
                            for j in range(4):
                                jj = nb * 4 + j
                                nc.sync.dma_start(
                                    out=v_sb[m][:, jj, :],
                                    in_=vflat[(half, m)][:,
                                                         j * 128:
                                                         (j + 1) * 128],
                                    transpose=True)

        # ---------------- attention quarter (software-pipelined) --------
        def make_attention(pst, pyt, espool, tpool):
            def attention_quarter(b, hf):
                t0 = b * T
                ai = b * 2 + hf
                pending = [None]

                def flush():
                    if pending[0] is not None:
                        pending[0]()
                        pending[0] = None

                for qb in (2 * hf, 2 * hf + 1):
                    for m in range(HC):
                        nk = 4 * (qb + 1)
                        q0 = t0 + qb * 512
                        ytp = pyt.tile([128, 512], F32, tag="yt",
                                       name="ytp")
                        es = {}

                        def emit_scores(i2, _m=m, _qb=qb, _q0=q0, _es=es):
                            stp = pst.tile([128, 1024], F32, tag="st",
                                           name="stp")
                            e = espool.tile([128, 1024], MMDT, tag="es",
                                            name="es")
                            for j in range(2):
                                i = i2 + j
                                sl = slice(j * 512, (j + 1) * 512)
                                nc.tensor.matmul(
                                    stp[:, sl],
                                    kTf[_m][:, t0 + i * 128:
                                            t0 + (i + 1) * 128],
                                    qTf[_m][:, _q0:_q0 + 512],
                                    start=True, stop=True)
                                q_off = i * 128 - _qb * 512
                                if q_off >= 0:
                                    nc.vector.tensor_add(
                                        stp[:, j * 512 + q_off:
                                            j * 512 + q_off + 128],
                                        stp[:, j * 512 + q_off:
                                            j * 512 + q_off + 128],
                                        negm_sb)
                                    if j == 0 and q_off > 0:
                                        nc.vector.memset(e[:, 0:q_off], 0.0)
                                    if j == 1 and q_off > 0:
                                        nc.vector.memset(
                                            stp[:, 512:512 + q_off], NEG)
                            _es[i2] = (stp, e)

                        emit_scores(0)
                        flush()
                        for i2 in range(0, nk, 2):
                            if i2 + 2 < nk:
                                emit_scores(i2 + 2)
                            stp, e = es.pop(i2)
                            q_off0 = i2 * 128 - qb * 512
                            lo = max(0, q_off0)
                            nc.scalar.activation(out=e[:, lo:],
                                                 in_=stp[:, lo:],
                                                 func=AF.Exp, scale=SCALE)
                            for j in range(2):
                                i = i2 + j
                                sl = slice(j * 512, (j + 1) * 512)
                                nc.tensor.matmul(
                                    ytp, v_sb[m][:, b * 16 + i, :],
                                    e[:, sl], start=(i == 0),
                                    stop=(i == nk - 1))

                        def tail(_m=m, _qb=qb, _ytp=ytp):
                            ystash = tpool.tile([128, 512], MMDT, tag="yst",
                                                name="ystash")
                            nc.vector.tensor_copy(ystash, _ytp)
                            sq = tpool.tile([128, 512], MMDT, tag="ysq",
                                            name="ysq")
                            nc.vector.tensor_mul(sq, _ytp, ystash)
                            ssy = pyt.tile([128, 512], F32, tag="yt",
                                           name="ssy")
                            nc.tensor.matmul(ssy, ones_r, sq,
                                             start=True, stop=True)
                            lny = tpool.tile([128, 512], F32, tag="lny",
                                             name="lny")
                            nc.scalar.activation(out=lny, in_=ssy,
                                                 func=AF.Ln,
                                                 scale=1.0 / float(D))
                            fy = tpool.tile([128, 512], F32, tag="fy",
                                            name="fy")
                            nc.scalar.activation(out=fy, in_=lny,
                                                 func=AF.Exp, scale=-0.5)
                            yf1 = tpool.tile([128, 512], F32, tag="yf1",
                                             name="yf1")
                            nc.vector.tensor_mul(yf1, ystash, fy)
                            yf = tpool.tile([128, 512], MMDT, tag="yf",
                                            name="yf")
                            nc.vector.tensor_mul(
                                yf, yf1,
                                sg_sb[_m][:, t0 + _qb * 512:
                                          t0 + (_qb + 1) * 512])
                            sh0 = 4 * (_qb % 2)
                            nc.sync.dma_start(
                                out=a2a_in[ai][sh0:sh0 + 4,
                                               _m * 128:(_m + 1) * 128,
                                               :].rearrange("s p t -> p s t"),
                                in_=yf)

                        pending[0] = tail
                flush()
                nc.gpsimd.collective_compute(
                    "AllToAll", ALU.bypass,
                    ins=[a2a_in[ai]], outs=[a2a_out[ai]],
                    replica_groups=[list(range(NCORES))],
                )
            return attention_quarter

        def make_final(pfo, fyp, fop, wo_sb):
            def final_chunk(ai):
                yg = fyp.tile([128, KT, 128], MMDT, tag="yg", name="yg")
                nc.sync.dma_start(
                    out=yg,
                    in_=a2a_out[ai].rearrange("r (h p) t -> p (r h) t",
                                              p=128))
                for cp in range(2):
                    fo = [pfo.tile([128, 512], F32, tag=f"fo{cc}",
                                   name=f"fo{cc}") for cc in range(2)]
                    for kd in range(KT):
                        for cc in range(2):
                            col = cp * 1024 + cc * 512
                            nc.tensor.matmul(
                                fo[cc], yg[:, kd, :],
                                wo_sb[:, kd, col:col + 512],
                                start=(kd == 0), stop=(kd == KT - 1))
                    for cc in range(2):
                        ot = fop.tile([128, 512], F32, tag="ot", name="ot")
                        nc.vector.tensor_copy(ot, fo[cc])
                        nc.sync.dma_start(
                            out=out_tok[ai, :, cp * 1024 + cc * 512:
                                        cp * 1024 + cc * 512 + 512],
                            in_=ot)
            return final_chunk

        # =================== phase schedule =============================
        with tc.tile_pool(name="weights", bufs=1) as wpool:
            wsb = {}
            for wn, w_ap in (("q", wq), ("k", wk), ("g", wg), ("v", wv)):
                w_t = wpool.tile([128, KT, DC], MMDT, tag=f"w{wn}",
                                 name=f"w{wn}")
                nc.sync.dma_start(out=w_t, in_=w_ap)
                wsb[wn] = w_t

            sweep_a()
            sweep_b([0, 1])
            # attention for batch 0 (overlaps nothing upstream, but its
            # collectives overlap the b1 sweep below)
            with tc.tile_pool(name="at_st", bufs=2, space="PSUM") as pst, \
                 tc.tile_pool(name="at_yt", bufs=2, space="PSUM") as pyt, \
                 tc.tile_pool(name="at_es", bufs=4) as espool, \
                 tc.tile_pool(name="at_t", bufs=3) as tpool:
                attention_quarter = make_attention(pst, pyt, espool, tpool)
                attention_quarter(0, 0)
                attention_quarter(0, 1)
            sweep_b([2, 3])

        # weights pool closed; attention for batch 1 + all finals
        with tc.tile_pool(name="at_st", bufs=2, space="PSUM") as pst, \
             tc.tile_pool(name="at_yt", bufs=2, space="PSUM") as pyt, \
             tc.tile_pool(name="fin_ps", bufs=1, space="PSUM") as pfo, \
             tc.tile_pool(name="at_es", bufs=4) as espool, \
             tc.tile_pool(name="at_t", bufs=3) as tpool, \
             tc.tile_pool(name="fin_w", bufs=1) as fwp, \
             tc.tile_pool(name="fin_y", bufs=2) as fyp, \
             tc.tile_pool(name="fin_o", bufs=2) as fop:
            wo_sb = fwp.tile([128, KT, HID], MMDT, tag="wo")
            nc.sync.dma_start(out=wo_sb, in_=wo)
            attention_quarter = make_attention(pst, pyt, espool, tpool)
            final_chunk = make_final(pfo, fyp, fop, wo_sb)
            attention_quarter(1, 0)
            final_chunk(0)
            final_chunk(1)
            attention_quarter(1, 1)
            final_chunk(2)
            final_chunk(3)
        stack.close()
    nc.compile()
    return nc


def _get_nc():
    global _CACHED_NC
    if _CACHED_NC is None:
        _CACHED_NC = _build_nc()
    return _CACHED_NC


def kernel(x, Wq, Wk, Wv, Wg, Wo, q_rms_w, k_rms_w, o_norm_w):
    global LAST_EXEC_TIME_NS
    import ml_dtypes
    npdt = ml_dtypes.bfloat16
    x = np.asarray(x, dtype=np.float32)
    Wq = np.asarray(Wq, dtype=np.float32)
    Wk = np.asarray(Wk, dtype=np.float32)
    Wv = np.asarray(Wv, dtype=np.float32)
    Wg = np.asarray(Wg, dtype=np.float32)
    Wo = np.asarray(Wo, dtype=np.float32)
    q_rms_w = np.asarray(q_rms_w, dtype=np.float32)
    k_rms_w = np.asarray(k_rms_w, dtype=np.float32)
    o_norm_w = np.asarray(o_norm_w, dtype=np.float32)

    xT = x.reshape(BT, HID).T          # [HID, BT]
    xt4 = np.ascontiguousarray(
        xT.reshape(KT, 128, BT // 1024, 1024).transpose(0, 2, 1, 3)).astype(npdt)
    # fold o_norm_w into Wo rows: (y*o_w) @ Wo == y @ (o_w[:,None]*Wo)
    wo_scaled = Wo * np.tile(o_norm_w, H)[:, None]
    wo_t = np.ascontiguousarray(
        wo_scaled.reshape(KT, 128, HID).transpose(1, 0, 2)).astype(npdt)

    inv = 1.0 / (10000.0 ** (np.arange(0, D, 2, dtype=np.float64) / D))
    pos = np.arange(T, dtype=np.float64)
    fr = pos[:, None] * inv[None, :]          # [T, 64]
    cosT = np.cos(fr).T.astype(np.float32)    # [64, T]
    sinT = np.sin(fr).T.astype(np.float32)
    cos2 = np.ascontiguousarray(np.vstack([cosT, cosT]))   # [128, T]
    # top half negated: rope becomes raw*cos + swap(raw)*sin' full-width
    sin2 = np.ascontiguousarray(np.vstack([-sinT, sinT]))

    kk, qq = np.meshgrid(np.arange(128), np.arange(128), indexing="ij")
    negm = np.where(kk <= qq, 0.0, NEG).astype(np.float32)
    ones128 = np.ones((128, 128), dtype=np.float32)

    in_maps = []
    for c in range(NCORES):
        csl = slice(c * DC, (c + 1) * DC)

        def wt(wmat):
            return np.ascontiguousarray(
                wmat[:, csl].reshape(KT, 128, DC).transpose(1, 0, 2)).astype(npdt)
        in_maps.append({
            "xT": xt4,
            "wq": wt(Wq),
            "wk": wt(Wk),
            "wv": wt(Wv),
            "wg": wt(Wg),
            "wo": wo_t,
            "cos2": cos2,
            "sin2": sin2,
            "negm": negm,
            "ones_in": ones128.astype(npdt),
            "qrw": np.ascontiguousarray(q_rms_w.reshape(128, 1)),
            "krw": np.ascontiguousarray(k_rms_w.reshape(128, 1)),
        })

    nc = _get_nc()
    trace = os.environ.get("KERNEL_TRACE", "0") == "1"
    res = run_bass_kernel_spmd(nc, in_maps, list(range(NCORES)), trace=trace)
    LAST_EXEC_TIME_NS = res.exec_time_ns

    out = np.empty((B, T, HID), dtype=np.float32)
    for c in range(NCORES):
        ot = res.results[c]["out_tok"]        # [4, 128, 2048]
        for b in range(B):
            for hf in range(2):
                t0 = hf * 1024 + c * 128
                out[b, t0:t0 + 128, :] = ot[b * 2 + hf]
    return out


# revision 7
# speedup vs baseline: 1.0650x; 1.0650x over previous
"""Trainium2 Bass kernel for a dense transformer block (QKV+gate proj, RoPE,
QK-RMSNorm, causal SDPA, output-RMSNorm + SiLU gate, output projection).

Sharding: tensor-parallel over heads across 8 NeuronCores (2 heads/core) for
projections+attention; token-parallel output projection with replicated Wo,
exchanged via 4 small AllToAlls (one per batch-half).

Schedule (engine FIFOs execute in trace order, so phases are interleaved):
  sweep A (q,k + fused rope/rms post)
  sweep B pairs 0,1  (gate+v for batch 0)
  attention b0 half0 + A2A, half1 + A2A     <- overlaps sweep-B b1 below
  sweep B pairs 2,3  (gate+v for batch 1)
  attention b1 half0 + A2A + finals(b0), half1 + A2A + finals(b1)
Attention is software-pipelined (scores i2+1 issued before PV i2) and each
(qb,m) tail's PSUM matmul is deferred behind the next iteration's scores.

Numerics: all matmuls bf16 (f32 accumulate). The output-RMSNorm eps term is
dropped (validated <3e-3 output error), removing all softmax-denominator
matmuls. rsqrt and silu are computed via exp/ln so the whole kernel uses a
single ACT table set (natural_log_exp_and_others).
"""

import os
import sys

for _p in ("/opt/trn_rl_repo", "/root/.axon_site/_ro/trn_rl_repo"):
    if os.path.isdir(_p) and _p not in sys.path:
        sys.path.insert(0, _p)

import numpy as np

import concourse.bass as bass
import concourse.mybir as mybir
from concourse import bacc
from concourse.bass_utils import run_bass_kernel_spmd
from concourse.tile import TileContext

B, T, HID = 2, 2048, 2048
H, D = 16, 128
NCORES = 8
HC = H // NCORES          # heads per core = 2
DC = HC * D               # 256 head-dims per core
BT = B * T                # 4096 tokens
KT = HID // 128           # 16 contraction tiles
SCALE = 1.0 / float(np.sqrt(D))
NEG = -3.0e38

F32 = mybir.dt.float32
BF16 = mybir.dt.bfloat16
AF = mybir.ActivationFunctionType
ALU = mybir.AluOpType

MMDT = BF16

LAST_EXEC_TIME_NS = None
_CACHED_NC = None


class _Bacc(bacc.Bacc):
    """Bacc with an ACT-table-set preference: serve Exp and Ln from the
    combined natural_log_exp_and_others set so alternating Ln/Exp chains
    don't thrash table loads."""

    def insert_act_table_loads(self):
        import bass_rust as _bass_rust
        from concourse.hw_specs import get_activation_tables
        has_activation = any(
            isinstance(i, mybir.InstActivation)
            for b in self.main_func.blocks
            for i in b.instructions
        )
        if not has_activation:
            return
        AFT = mybir.ActivationFunctionType
        tables = []
        for name, fns in get_activation_tables(self.m.arch).items():
            if name != "natural_log_exp_and_others":
                fns = fns - {AFT.Exp, AFT.Ln}
            tables.append((name, fns))
        _bass_rust.insert_act_table_loads(self, tables)


def _build_nc():
    nc = _Bacc("TRN2", target_bir_lowering=False, debug=False,
               num_devices=NCORES)

    xT = nc.dram_tensor("xT", [KT, BT // 1024, 128, 1024], MMDT,
                        kind="ExternalInput").ap()
    wq = nc.dram_tensor("wq", [128, KT, DC], MMDT, kind="ExternalInput").ap()
    wk = nc.dram_tensor("wk", [128, KT, DC], MMDT, kind="ExternalInput").ap()
    wv = nc.dram_tensor("wv", [128, KT, DC], MMDT, kind="ExternalInput").ap()
    wg = nc.dram_tensor("wg", [128, KT, DC], MMDT, kind="ExternalInput").ap()
    # full (replicated) Wo with o_norm folded: [128, kd-tile, 2048 cols]
    wo = nc.dram_tensor("wo", [128, KT, HID], MMDT, kind="ExternalInput").ap()
    cos2 = nc.dram_tensor("cos2", [128, T], F32, kind="ExternalInput").ap()
    sin2 = nc.dram_tensor("sin2", [128, T], F32, kind="ExternalInput").ap()
    negm = nc.dram_tensor("negm", [128, 128], F32, kind="ExternalInput").ap()
    ones_in = nc.dram_tensor("ones_in", [128, 128], MMDT,
                             kind="ExternalInput").ap()
    qrw = nc.dram_tensor("qrw", [128, 1], F32, kind="ExternalInput").ap()
    krw = nc.dram_tensor("krw", [128, 1], F32, kind="ExternalInput").ap()

    # output: per (b, half) a [128 tok, 2048] f32 chunk
    out_tok = nc.dram_tensor("out_tok", [B * 2, 128, HID], F32,
                             kind="ExternalOutput").ap()

    # AllToAll buffers: per (b, half): [8 shards(128 tok), 256 hd, 128 tok]
    a2a_in = [nc.dram_tensor(f"a2a_in{i}", [NCORES, DC, 128], MMDT).ap()
              for i in range(B * 2)]
    a2a_out = [nc.dram_tensor(f"a2a_out{i}", [NCORES, DC, 128], MMDT).ap()
               for i in range(B * 2)]

    tc = TileContext(nc)
    from contextlib import ExitStack
    stack = ExitStack()
    with tc:
        const = stack.enter_context(tc.tile_pool(name="const", bufs=1))
        ones_r = const.tile([128, 128], MMDT)
        nc.scalar.dma_start(out=ones_r, in_=ones_in)
        negm_sb = const.tile([128, 128], F32)
        nc.scalar.dma_start(out=negm_sb, in_=negm)
        onesb = const.tile([128, 1], F32)
        nc.vector.memset(onesb, 1.0)

        persist = stack.enter_context(tc.tile_pool(name="persist", bufs=1))
        qTf = [persist.tile([128, BT], MMDT, tag=f"qTf{m}", name=f"qTf{m}")
               for m in range(HC)]
        kTf = [persist.tile([128, BT], MMDT, tag=f"kTf{m}", name=f"kTf{m}")
               for m in range(HC)]
        v_sb = [persist.tile([128, BT // 128, 128], MMDT, tag=f"v{m}",
                             name=f"v{m}") for m in range(HC)]
        sg_sb = [persist.tile([128, BT], MMDT, tag=f"sg{m}", name=f"sg{m}")
                 for m in range(HC)]

        # ---------------- sweep A: q, k + fused post --------------------
        def sweep_a():
            with tc.tile_pool(name="swA_x", bufs=5) as xpool, \
                 tc.tile_pool(name="swA_ps", bufs=1, space="PSUM") as pps, \
                 tc.tile_pool(name="swA_ss", bufs=2, space="PSUM") as pss, \
                 tc.tile_pool(name="swA_t", bufs=1) as tpool, \
                 tc.tile_pool(name="ropec", bufs=1) as rp:
                cos_sb = rp.tile([128, T], F32)
                nc.sync.dma_start(out=cos_sb, in_=cos2)
                sin_sb = rp.tile([128, T], F32)
                nc.sync.dma_start(out=sin_sb, in_=sin2)
                qrw_sb = rp.tile([128, 1], F32)
                nc.sync.dma_start(out=qrw_sb, in_=qrw)
                krw_sb = rp.tile([128, 1], F32)
                nc.sync.dma_start(out=krw_sb, in_=krw)
                wsc = [qrw_sb, krw_sb]

                for nbb in range(BT // 1024):
                    xch = []
                    for c in range(4):
                        xc = xpool.tile([128, 4, 1024], MMDT, tag="xc",
                                        name="xc")
                        nc.sync.dma_start(
                            out=xc,
                            in_=xT[4 * c:4 * c + 4, nbb, :, :].rearrange(
                                "k p t -> p k t"))
                        xch.append(xc)
                    for half in range(2):
                        nb = 2 * nbb + half
                        c0 = nb * 512
                        ct0 = c0 % T
                        hs = slice(half * 512, (half + 1) * 512)
                        mms = [(mi, m) for mi in range(2) for m in range(HC)]
                        ps = {}
                        for mi, m in mms:
                            ps[(mi, m)] = pps.tile(
                                [128, 512], F32, tag=f"pp{mi}{m}",
                                name=f"pp{mi}{m}",
                                bufs=(2 if mi == 0 else 1))
                        for k in range(KT):
                            xk = xch[k // 4][:, k % 4, hs]
                            for mi, wn in enumerate(("q", "k")):
                                for m in range(HC):
                                    nc.tensor.matmul(
                                        ps[(mi, m)],
                                        wsb[wn][:, k, m * 128:(m + 1) * 128],
                                        xk, start=(k == 0),
                                        stop=(k == KT - 1))
                        dests = [qTf, kTf]
                        for mi, m in mms:
                            nc.vector.tensor_copy(
                                dests[mi][m][:, c0:c0 + 512], ps[(mi, m)])
                        # post: rms factor + rope in place; long-dependency
                        # ops go at the END of the DVE stream
                        raws = {k2: dests[k2[0]][k2[1]][:, c0:c0 + 512]
                                for k2 in mms}
                        facs, ros = {}, {}
                        for mi, m in mms:
                            raw = raws[(mi, m)]
                            sq = tpool.tile([128, 512], MMDT, tag="sq",
                                            name="sq", bufs=4)
                            nc.vector.tensor_mul(sq, raw, raw)
                            ss = pss.tile([128, 512], F32, tag="ss",
                                          name="ss")
                            nc.tensor.matmul(ss, ones_r, sq,
                                             start=True, stop=True)
                            lnt = tpool.tile([128, 512], F32, tag="lnt",
                                             name="lnt", bufs=4)
                            nc.scalar.activation(out=lnt, in_=ss, func=AF.Ln,
                                                 scale=1.0 / float(D))
                            fac = tpool.tile([128, 512], F32, tag="fac",
                                             name="fac", bufs=4)
                            nc.scalar.activation(out=fac, in_=lnt,
                                                 func=AF.Exp, scale=-0.5)
                            facs[(mi, m)] = fac
                        cc = cos_sb[:, ct0:ct0 + 512]
                        ssn = sin_sb[:, ct0:ct0 + 512]
                        for mi, m in mms:
                            raw = raws[(mi, m)]
                            # sin_sb top half pre-negated: full-width rope
                            sw = tpool.tile([128, 512], MMDT, tag="sw",
                                            name="sw", bufs=4)
                            nc.scalar.dma_start(out=sw[0:64, :],
                                                in_=raw[64:128, :])
                            nc.scalar.dma_start(out=sw[64:128, :],
                                                in_=raw[0:64, :])
                            u = tpool.tile([128, 512], F32, tag="u",
                                           name="u", bufs=4)
                            w2 = tpool.tile([128, 512], F32, tag="w2",
                                            name="w2", bufs=4)
                            nc.vector.tensor_mul(u, raw, cc)
                            nc.gpsimd.tensor_mul(w2, sw, ssn)
                            ro = tpool.tile([128, 512], F32, tag="ro",
                                            name="ro", bufs=4)
                            nc.gpsimd.tensor_add(ro, u, w2)
                            ros[(mi, m)] = ro
                        for mi, m in mms:
                            nc.vector.scalar_tensor_tensor(
                                out=raws[(mi, m)], in0=ros[(mi, m)],
                                scalar=wsc[mi], in1=facs[(mi, m)],
                                op0=ALU.mult, op1=ALU.mult)

        # ---------------- sweep B (gate, v) for a pair list -------------
        def sweep_b(nbb_list):
            with tc.tile_pool(name="swB_x", bufs=5) as xpool, \
                 tc.tile_pool(name="swB_ps", bufs=1, space="PSUM") as pps, \
                 tc.tile_pool(name="swB_t", bufs=1) as tpool:
                for nbb in nbb_list:
                    xch = []
                    for c in range(4):
                        xc = xpool.tile([128, 4, 1024], MMDT, tag="xc",
                                        name="xc")
                        nc.sync.dma_start(
                            out=xc,
                            in_=xT[4 * c:4 * c + 4, nbb, :, :].rearrange(
                                "k p t -> p k t"))
                        xch.append(xc)
                    # pair-interleaved: one weight load feeds both halves
                    ps = {}
                    for mi in range(2):
                        for m in range(HC):
                            for half in range(2):
                                ps[(mi, m, half)] = pps.tile(
                                    [128, 512], F32, tag=f"pp{mi}{m}{half}",
                                    name=f"pp{mi}{m}{half}", bufs=1)
                    for k in range(KT):
                        for mi, wn in enumerate(("g", "v")):
                            for m in range(HC):
                                for half in range(2):
                                    hs = slice(half * 512, (half + 1) * 512)
                                    nc.tensor.matmul(
                                        ps[(mi, m, half)],
                                        wsb[wn][:, k, m * 128:(m + 1) * 128],
                                        xch[k // 4][:, k % 4, hs],
                                        start=(k == 0), stop=(k == KT - 1))
                    # all evacuations first, ACT-dependent chains after
                    vflat = {}
                    graws = {}
                    for half in range(2):
                        c0 = (2 * nbb + half) * 512
                        for m in range(HC):
                            graw = tpool.tile([128, 512], F32, tag="graw",
                                              name="graw", bufs=6)
                            nc.vector.tensor_copy(graw, ps[(0, m, half)])
                            graws[(half, m)] = graw
                            vf = tpool.tile([128, 512], MMDT, tag="vf",
                                            name="vf", bufs=6)
                            nc.vector.tensor_copy(vf, ps[(1, m, half)])
                            vflat[(half, m)] = vf
                    for half in range(2):
                        nb = 2 * nbb + half
                        c0 = nb * 512
                        for m in range(HC):
                            graw = graws[(half, m)]
                            e1 = tpool.tile([128, 512], F32, tag="e1",
                                            name="e1", bufs=3)
                            nc.scalar.activation(out=e1, in_=graw,
                                                 func=AF.Exp, scale=-1.0)
                            l1 = tpool.tile([128, 512], F32, tag="l1",
                                            name="l1", bufs=3)
                            nc.scalar.activation(out=l1, in_=e1, func=AF.Ln,
                                                 scale=1.0, bias=onesb)
                            s1 = tpool.tile([128, 512], F32, tag="s1",
                                            name="s1", bufs=3)
                            nc.scalar.activation(out=s1, in_=l1,
                                                 func=AF.Exp, scale=-1.0)
                            nc.vector.tensor_mul(
                                sg_sb[m][:, c0:c0 + 512], graw, s1)
                            for j in range(4):
                                jj = nb * 4 + j
                                nc.sync.dma_start(
                                    out=v_sb[m][:, jj, :],
                                    in_=vflat[(half, m)][:,
                                                         j * 128:
                                                         (j + 1) * 128],
                                    transpose=True)

        # ---------------- attention quarter (software-pipelined) --------
        def make_attention(pst, pyt, espool, tpool):
            def attention_quarter(b, hf):
                t0 = b * T
                ai = b * 2 + hf
                pending = [None]

                def flush():
                    if pending[0] is not None:
                        pending[0]()
                        pending[0] = None

                for qb in (2 * hf, 2 * hf + 1):
                    for m in range(HC):
                        nk = 4 * (qb + 1)
                        q0 = t0 + qb * 512
                        ytp = pyt.tile([128, 512], F32, tag="yt",
                                       name="ytp")
                        es = {}

                        def emit_scores(i2, _m=m, _qb=qb, _q0=q0, _es=es):
                            stp = pst.tile([128, 1024], F32, tag="st",
                                           name="stp")
                            e = espool.tile([128, 1024], MMDT, tag="es",
                                            name="es")
                            for j in range(2):
                                i = i2 + j
                                sl = slice(j * 512, (j + 1) * 512)
                                nc.tensor.matmul(
                                    stp[:, sl],
                                    kTf[_m][:, t0 + i * 128:
                                            t0 + (i + 1) * 128],
                                    qTf[_m][:, _q0:_q0 + 512],
                                    start=True, stop=True)
                                q_off = i * 128 - _qb * 512
                                if q_off >= 0:
                                    nc.vector.tensor_add(
                                        stp[:, j * 512 + q_off:
                                            j * 512 + q_off + 128],
                                        stp[:, j * 512 + q_off:
                                            j * 512 + q_off + 128],
                                        negm_sb)
                                    if j == 0 and q_off > 0:
                                        nc.vector.memset(e[:, 0:q_off], 0.0)
                                    if j == 1 and q_off > 0:
                                        nc.vector.memset(
                                            stp[:, 512:512 + q_off], NEG)
                            _es[i2] = (stp, e)

                        emit_scores(0)
                        flush()
                        for i2 in range(0, nk, 2):
                            if i2 + 2 < nk:
                                emit_scores(i2 + 2)
                            stp, e = es.pop(i2)
                            q_off0 = i2 * 128 - qb * 512
                            lo = max(0, q_off0)
                            nc.scalar.activation(out=e[:, lo:],
                                                 in_=stp[:, lo:],
                                                 func=AF.Exp, scale=SCALE)
                            for j in range(2):
                                i = i2 + j
                                sl = slice(j * 512, (j + 1) * 512)
                                nc.tensor.matmul(
                                    ytp, v_sb[m][:, b * 16 + i, :],
                                    e[:, sl], start=(i == 0),
                                    stop=(i == nk - 1))

                        def tail(_m=m, _qb=qb, _ytp=ytp):
                            ystash = tpool.tile([128, 512], MMDT, tag="yst",
                                                name="ystash")
                            nc.vector.tensor_copy(ystash, _ytp)
                            sq = tpool.tile([128, 512], MMDT, tag="ysq",
                                            name="ysq")
                            nc.vector.tensor_mul(sq, _ytp, ystash)
                            ssy = pyt.tile([128, 512], F32, tag="yt",
                                           name="ssy")
                            nc.tensor.matmul(ssy, ones_r, sq,
                                             start=True, stop=True)
                            lny = tpool.tile([128, 512], F32, tag="lny",
                                             name="lny")
                            nc.scalar.activation(out=lny, in_=ssy,
                                                 func=AF.Ln,
                                                 scale=1.0 / float(D))
                            fy = tpool.tile([128, 512], F32, tag="fy",
                                            name="fy")
                            nc.scalar.activation(out=fy, in_=lny,
                                                 func=AF.Exp, scale=-0.5)
                            yf1 = tpool.tile([128, 512], F32, tag="yf1",
                                             name="yf1")
                            nc.vector.tensor_mul(yf1, ystash, fy)
                            yf = tpool.tile([128, 512], MMDT, tag="yf",
                                            name="yf")
                            nc.vector.tensor_mul(
                                yf, yf1,
                                sg_sb[_m][:, t0 + _qb * 512:
                                          t0 + (_qb + 1) * 512])
                            sh0 = 4 * (_qb % 2)
                            nc.sync.dma_start(
                                out=a2a_in[ai][sh0:sh0 + 4,
                                               _m * 128:(_m + 1) * 128,
                                               :].rearrange("s p t -> p s t"),
                                in_=yf)

                        pending[0] = tail
                flush()
                nc.gpsimd.collective_compute(
                    "AllToAll", ALU.bypass,
                    ins=[a2a_in[ai]], outs=[a2a_out[ai]],
                    replica_groups=[list(range(NCORES))],
                )
            return attention_quarter

        def make_final(pfo, fyp, fop, wo_sb):
            def final_chunk(ai):
                yg = fyp.tile([128, KT, 128], MMDT, tag="yg", name="yg")
                nc.sync.dma_start(
                    out=yg,
                    in_=a2a_out[ai].rearrange("r (h p) t -> p (r h) t",
                                              p=128))
                for cp in range(2):
                    fo = [pfo.tile([128, 512], F32, tag=f"fo{cc}",
                                   name=f"fo{cc}") for cc in range(2)]
                    for kd in range(KT):
                        for cc in range(2):
                            col = cp * 1024 + cc * 512
                            nc.tensor.matmul(
                                fo[cc], yg[:, kd, :],
                                wo_sb[:, kd, col:col + 512],
                                start=(kd == 0), stop=(kd == KT - 1))
                    for cc in range(2):
                        ot = fop.tile([128, 512], F32, tag="ot", name="ot")
                        nc.vector.tensor_copy(ot, fo[cc])
                        nc.sync.dma_start(
                            out=out_tok[ai, :, cp * 1024 + cc * 512:
                                        cp * 1024 + cc * 512 + 512],
                            in_=ot)
            return final_chunk

        # =================== phase schedule =============================
        with tc.tile_pool(name="weights", bufs=1) as wpool:
            wsb = {}
            for wn, w_ap in (("q", wq), ("k", wk), ("g", wg), ("v", wv)):
                w_t = wpool.tile([128, KT, DC], MMDT, tag=f"w{wn}",
                                 name=f"w{wn}")
                nc.sync.dma_start(out=w_t, in_=w_ap)
                wsb[wn] = w_t

            sweep_a()
            sweep_b([0, 1, 2, 3])

        # weights pool closed; attention + finals (delay-one-quarter)
        with tc.tile_pool(name="at_st", bufs=2, space="PSUM") as pst, \
             tc.tile_pool(name="at_yt", bufs=2, space="PSUM") as pyt, \
             tc.tile_pool(name="fin_ps", bufs=1, space="PSUM") as pfo, \
             tc.tile_pool(name="at_es", bufs=4) as espool, \
             tc.tile_pool(name="at_t", bufs=3) as tpool, \
             tc.tile_pool(name="fin_w", bufs=1) as fwp, \
             tc.tile_pool(name="fin_y", bufs=2) as fyp, \
             tc.tile_pool(name="fin_o", bufs=2) as fop:
            wo_sb = fwp.tile([128, KT, HID], MMDT, tag="wo")
            nc.sync.dma_start(out=wo_sb, in_=wo)
            attention_quarter = make_attention(pst, pyt, espool, tpool)
            final_chunk = make_final(pfo, fyp, fop, wo_sb)
            attention_quarter(0, 0)
            attention_quarter(0, 1)
            final_chunk(0)
            attention_quarter(1, 0)
            final_chunk(1)
            attention_quarter(1, 1)
            final_chunk(2)
            final_chunk(3)
        stack.close()
    nc.compile()
    return nc


def _get_nc():
    global _CACHED_NC
    if _CACHED_NC is None:
        _CACHED_NC = _build_nc()
    return _CACHED_NC


def kernel(x, Wq, Wk, Wv, Wg, Wo, q_rms_w, k_rms_w, o_norm_w):
    global LAST_EXEC_TIME_NS
    import ml_dtypes
    npdt = ml_dtypes.bfloat16
    x = np.asarray(x, dtype=np.float32)
    Wq = np.asarray(Wq, dtype=np.float32)
    Wk = np.asarray(Wk, dtype=np.float32)
    Wv = np.asarray(Wv, dtype=np.float32)
    Wg = np.asarray(Wg, dtype=np.float32)
    Wo = np.asarray(Wo, dtype=np.float32)
    q_rms_w = np.asarray(q_rms_w, dtype=np.float32)
    k_rms_w = np.asarray(k_rms_w, dtype=np.float32)
    o_norm_w = np.asarray(o_norm_w, dtype=np.float32)

    xT = x.reshape(BT, HID).T          # [HID, BT]
    xt4 = np.ascontiguousarray(
        xT.reshape(KT, 128, BT // 1024, 1024).transpose(0, 2, 1, 3)).astype(npdt)
    # fold o_norm_w into Wo rows: (y*o_w) @ Wo == y @ (o_w[:,None]*Wo)
    wo_scaled = Wo * np.tile(o_norm_w, H)[:, None]
    wo_t = np.ascontiguousarray(
        wo_scaled.reshape(KT, 128, HID).transpose(1, 0, 2)).astype(npdt)

    inv = 1.0 / (10000.0 ** (np.arange(0, D, 2, dtype=np.float64) / D))
    pos = np.arange(T, dtype=np.float64)
    fr = pos[:, None] * inv[None, :]          # [T, 64]
    cosT = np.cos(fr).T.astype(np.float32)    # [64, T]
    sinT = np.sin(fr).T.astype(np.float32)
    cos2 = np.ascontiguousarray(np.vstack([cosT, cosT]))   # [128, T]
    # top half negated: rope becomes raw*cos + swap(raw)*sin' full-width
    sin2 = np.ascontiguousarray(np.vstack([-sinT, sinT]))

    kk, qq = np.meshgrid(np.arange(128), np.arange(128), indexing="ij")
    negm = np.where(kk <= qq, 0.0, NEG).astype(np.float32)
    ones128 = np.ones((128, 128), dtype=np.float32)

    in_maps = []
    for c in range(NCORES):
        csl = slice(c * DC, (c + 1) * DC)

        def wt(wmat):
            return np.ascontiguousarray(
                wmat[:, csl].reshape(KT, 128, DC).transpose(1, 0, 2)).astype(npdt)
        in_maps.append({
            "xT": xt4,
            "wq": wt(Wq),
            "wk": wt(Wk),
            "wv": wt(Wv),
            "wg": wt(Wg),
            "wo": wo_t,
            "cos2": cos2,
            "sin2": sin2,
            "negm": negm,
            "ones_in": ones128.astype(npdt),
            "qrw": np.ascontiguousarray(q_rms_w.reshape(128, 1)),
            "krw": np.ascontiguousarray(k_rms_w.reshape(128, 1)),
        })

    nc = _get_nc()
    trace = os.environ.get("KERNEL_TRACE", "0") == "1"
    res = run_bass_kernel_spmd(nc, in_maps, list(range(NCORES)), trace=trace)
    LAST_EXEC_TIME_NS = res.exec_time_ns

    out = np.empty((B, T, HID), dtype=np.float32)
    for c in range(NCORES):
        ot = res.results[c]["out_tok"]        # [4, 128, 2048]
        for b in range(B):
            for hf in range(2):
                t0 = hf * 1024 + c * 128
                out[b, t0:t0 + 128, :] = ot[b * 2 + hf]
    return out
